# revision 27
# baseline (speedup 1.0000x reference)
"""Trainium2 Bass kernel for nn_AccentCLS: BN1d -> Linear -> BN1d -> cosine-sim vs template.

Self-contained: hardcodes shapes (B=32768, D=1024, H=256, C=16), shards the batch
across 8 NeuronCores, runs a single fused Bass/Tile kernel per core with two tiny
AllReduces for the global BatchNorm statistics, and gathers the full [B, C] output.

Math (exact reformulation of the reference):
  BN1: s1 = g1 / sqrt(var1 + eps), applied by folding into W:  W' = W * s1
       hc := h - mean(h) = x @ W'.T - W @ (mu1 * s1)     (bias b and beta1 cancel)
  BN2: var2 = E[hc^2] (mean(hc) == 0 exactly), s2 = g2 / sqrt(var2 + eps)
       h_bn = hc * s2 + beta2
  cos[b,c] = (hc.T @ (s2*t))[c,b] + sum_j beta2*t
             ----------------------------------- * 1/||t_c||
             sqrt(sum s2^2 hc^2 + 2 sum s2 b2 hc + sum b2^2)
"""

import os
import sys

for _p in ("/opt/trn_rl_repo", "/root/.axon_site/_ro/trn_rl_repo"):
    if os.path.isdir(_p) and _p not in sys.path:
        sys.path.insert(0, _p)

import numpy as np

import concourse.bass as bass
import concourse.mybir as mybir
import concourse.tile as tile
from concourse import bacc
from concourse.bass_utils import run_bass_kernel_spmd
from concourse.masks import make_identity

N_CORES = 8
P = 128
B_GLOB = 32768
BSH = B_GLOB // N_CORES  # 4096 rows per core
D = 1024
H = 256
C = 16
K = D // P    # 8 contraction chunks
M = H // P    # 2 hidden chunks
NT = BSH // P  # 32 batch tiles of 128
NB = BSH // 512  # 8 batch chunks of 512
EPS = 1e-5

F32 = mybir.dt.float32
F32R = mybir.dt.float32r
BF16 = mybir.dt.bfloat16
AF = mybir.ActivationFunctionType
ALU = mybir.AluOpType

_CACHED = {}


def _build():
    nc = bacc.Bacc("TRN2", target_bir_lowering=False, debug=False, num_devices=N_CORES)

    x_ap = nc.dram_tensor("x", [BSH, D], F32, kind="ExternalInput").ap()
    g1_ap = nc.dram_tensor("bn1_gamma", [D], F32, kind="ExternalInput").ap()
    w_ap = nc.dram_tensor("W", [H, D], F32, kind="ExternalInput").ap()
    g2_ap = nc.dram_tensor("bn2_gamma", [H], F32, kind="ExternalInput").ap()
    b2_ap = nc.dram_tensor("bn2_beta", [H], F32, kind="ExternalInput").ap()
    t_ap = nc.dram_tensor("template", [H, C], F32, kind="ExternalInput").ap()
    out_ap = nc.dram_tensor("out", [BSH, C], F32, kind="ExternalOutput").ap()

    with tile.TileContext(nc) as tc:
        _emit(nc, tc, x_ap, g1_ap, w_ap, g2_ap, b2_ap, t_ap, out_ap)
    nc.compile()
    return nc


def _emit(nc, tc, x_ap, g1_ap, w_ap, g2_ap, b2_ap, t_ap, out_ap, phases="full"):
    with (
        tc.tile_pool(name="persist", bufs=1) as pers,
        tc.tile_pool(name="loadp", bufs=4) as loadp,
        tc.tile_pool(name="roll", bufs=2) as roll,
        tc.tile_pool(name="small", bufs=1) as small,
        tc.tile_pool(name="dram", bufs=1, space="DRAM") as dram,
    ):
        # ---- resident tensors ----
        xT = pers.tile([P, K, BSH], BF16, name="xT")          # x.T  (d-chunk, batch)
        hc = [pers.tile([P, BSH], BF16, name=f"hc{m}") for m in range(M)]
        hcsq = [pers.tile([P, BSH], BF16, name=f"hcsq{m}") for m in range(M)]
        wT = pers.tile([P, K, H], F32, name="wT")             # W.T
        wpT = pers.tile([P, K, H], BF16, name="wpT")          # (W*s1).T
        ident = small.tile([P, P], BF16, name="identb")
        make_identity(nc, ident[:])
        identf = small.tile([P, P], F32, name="identf")
        make_identity(nc, identf[:])
        ident16 = small.tile([C, C], F32, name="ident16")
        make_identity(nc, ident16[:])
        eps_t = small.tile([P, 1], F32, name="eps")
        nc.vector.memset(eps_t[:], EPS)

        # ---- params (tiny DMAs; issued up front) ----
        g1_sb = small.tile([P, K], F32, name="g1")
        nc.sync.dma_start(g1_sb[:], g1_ap.rearrange("(k p) -> p k", p=P))
        g2_sb = small.tile([P, M], F32, name="g2")
        nc.sync.dma_start(g2_sb[:], g2_ap.rearrange("(m p) -> p m", p=P))
        b2_sb = small.tile([P, M], F32, name="b2")
        nc.sync.dma_start(b2_sb[:], b2_ap.rearrange("(m p) -> p m", p=P))
        t_sb = small.tile([P, M, C], F32, name="tmpl")
        nc.sync.dma_start(t_sb[:], t_ap.rearrange("(m p) c -> p m c", p=P))

        kap = small.tile([C, 1], F32, name="kap")        # sum_j beta2 * t[:, c]
        bsq = small.tile([1, 1], F32, name="bsq")        # sum_j beta2^2
        ntsq = small.tile([C, C], F32, name="ntsq")
        nt1 = small.tile([C, 1], F32, name="nt1")
        invnt = small.tile([C, 1], F32, name="invnt")
        kapi = small.tile([C, 1], F32, name="kapi")
        with tc.tile_pool(name="psumS", bufs=1, space="PSUM") as psumS:
            pk = psumS.tile([C, 1], F32, name="pk")
            for m in range(M):
                nc.tensor.matmul(pk[:], lhsT=t_sb[:, m, :], rhs=b2_sb[:, m:m + 1],
                                 start=(m == 0), stop=(m == M - 1))
            nc.scalar.copy(kap[:], pk[:])
            pb = psumS.tile([1, 1], F32, name="pb")
            for m in range(M):
                nc.tensor.matmul(pb[:], lhsT=b2_sb[:, m:m + 1], rhs=b2_sb[:, m:m + 1],
                                 start=(m == 0), stop=(m == M - 1))
            nc.scalar.copy(bsq[:], pb[:])
            pn = psumS.tile([C, C], F32, name="pn")
            for m in range(M):
                nc.tensor.matmul(pn[:], lhsT=t_sb[:, m, :], rhs=t_sb[:, m, :],
                                 start=(m == 0), stop=(m == M - 1))
            nc.vector.tensor_tensor(ntsq[:], pn[:], ident16[:], ALU.mult)
        nc.vector.tensor_reduce(
            out=nt1[:], in_=ntsq[:], axis=mybir.AxisListType.X, op=ALU.add)
        nc.scalar.activation(nt1[:], nt1[:], AF.Sqrt)
        nc.vector.reciprocal(invnt[:], nt1[:])
        nc.vector.tensor_mul(kapi[:], kap[:], invnt[:])

        # ---- W prep: load + PE-transpose (fp32) ----
        w_nat = [small.tile([P, D], F32, name=f"wnat{m}") for m in range(M)]
        for m in range(M):
            nc.sync.dma_start(w_nat[m][:], w_ap[m * P:(m + 1) * P, :])

        with tc.tile_pool(name="psumW", bufs=2, space="PSUM") as psumW:
            for m in range(M):
                for k in range(K):
                    pw = psumW.tile([P, P], F32, name="wps")
                    nc.tensor.transpose(pw[:], w_nat[m][:, k * P:(k + 1) * P], identf[:])
                    nc.vector.tensor_copy(wT[:, k, m * P:(m + 1) * P], pw[:])

        # ---- phase A: half-split cast-loads (d-chunks 0-3, then 4-7) ----
        # batch-column permutation: pipeline column 128*(4g+j)+p <-> batch row
        # 32p+4g+j; d split into two 512-wide halves so half-0's stats can
        # AllReduce while half-1 is still loading.
        stats_raw = pers.tile([P, K, NB, 6], F32, name="straw")
        x_h = x_ap.rearrange("(p g j) (h dc) -> h g p j dc", g=NT // 4, j=4, h=2)
        mv_all = small.tile([P, K, 2], F32, name="mvall")
        stats_loc = small.tile([P, 2 * K], F32, name="stloc")
        msq = small.tile([P, K], F32, name="msq")
        stats_g = small.tile([P, 2 * K], F32, name="stg")
        mu = small.tile([P, K], F32, name="mu")
        var1 = small.tile([P, K], F32, name="var1")
        s1 = small.tile([P, K], F32, name="s1")
        ms = small.tile([P, K], F32, name="ms")
        ar1i = [dram.tile([P, K], F32, name=f"ar1i{h}") for h in range(2)]
        ar1o = [dram.tile([P, K], F32, name=f"ar1o{h}") for h in range(2)]
        with tc.tile_pool(name="psumA", bufs=2, space="PSUM") as psumA:
            for h in range(2):
                hs = slice(4 * h, 4 * (h + 1))
                for g in range(NT // 4):
                    xn = loadp.tile([P, 4, 512], BF16, name="xnat")
                    nc.gpsimd.dma_start(out=xn[:], in_=x_h[h, g])
                    tps = psumA.tile([P, 4, 4, P], BF16, name="tpa")
                    for j in range(4):
                        for kk in range(4):
                            nc.tensor.transpose(
                                tps[:, kk, j, :], xn[:, j, kk * P:(kk + 1) * P],
                                ident[:])
                    nc.scalar.copy(
                        xT[:, hs, 512 * g:512 * (g + 1)],
                        tps.rearrange("p a b c -> p a (b c)"))
                    for kk in range(4):
                        k = 4 * h + kk
                        nc.vector.bn_stats(
                            out=stats_raw[:, k, g, :],
                            in_=xT[:, k, 512 * g:512 * (g + 1)])
                # this half's local sums -> AllReduce (overlaps the other half)
                for kk in range(4):
                    k = 4 * h + kk
                    nc.vector.bn_aggr(out=mv_all[:, k, :], in_=stats_raw[:, k, :, :])
                nc.vector.tensor_scalar_mul(
                    stats_loc[:, 4 * h:4 * h + 4], mv_all[:, hs, 0], float(BSH))
                nc.vector.tensor_mul(msq[:, hs], mv_all[:, hs, 0], mv_all[:, hs, 0])
                nc.vector.tensor_add(msq[:, hs], mv_all[:, hs, 1], msq[:, hs])
                nc.vector.tensor_scalar_mul(
                    stats_loc[:, K + 4 * h:K + 4 * h + 4], msq[:, hs], float(BSH))
                if phases == "A" and h == 1:
                    nc.sync.dma_start(out_ap[0:P, 0:C], stats_loc[:, 0:C])
                    return
                nc.sync.dma_start(ar1i[h][:, 0:4], stats_loc[:, 4 * h:4 * h + 4])
                nc.sync.dma_start(
                    ar1i[h][:, 4:8], stats_loc[:, K + 4 * h:K + 4 * h + 4])
                nc.gpsimd.collective_compute(
                    "AllReduce", ALU.add,
                    replica_groups=[list(range(N_CORES))],
                    ins=[ar1i[h].opt()], outs=[ar1o[h].opt()],
                )
                nc.sync.dma_start(stats_g[:, 4 * h:4 * h + 4], ar1o[h][:, 0:4])
                nc.sync.dma_start(
                    stats_g[:, K + 4 * h:K + 4 * h + 4], ar1o[h][:, 4:8])

                # BN1 scale for this half: s1 = g1*rsqrt(var+eps), ms = mu*s1
                nc.vector.tensor_scalar_mul(
                    mu[:, hs], stats_g[:, 4 * h:4 * h + 4], 1.0 / B_GLOB)
                nc.vector.tensor_scalar_mul(
                    var1[:, hs], stats_g[:, K + 4 * h:K + 4 * h + 4], 1.0 / B_GLOB)
                nc.vector.tensor_mul(s1[:, hs], mu[:, hs], mu[:, hs])
                nc.vector.tensor_tensor(var1[:, hs], var1[:, hs], s1[:, hs],
                                        ALU.subtract)
                nc.scalar.activation(var1[:, hs], var1[:, hs], AF.Sqrt, bias=eps_t[:])
                nc.vector.reciprocal(var1[:, hs], var1[:, hs])
                nc.vector.tensor_mul(s1[:, hs], var1[:, hs], g1_sb[:, hs])
                nc.vector.tensor_mul(ms[:, hs], mu[:, hs], s1[:, hs])
                # W' = W * s1 for this half's chunks (bf16)
                for kk in range(4):
                    k = 4 * h + kk
                    nc.vector.tensor_scalar_mul(
                        wpT[:, k, :], wT[:, k, :], s1[:, k:k + 1])

        # c = -W @ (mu*s1)
        c_sb = small.tile([P, M], F32, name="csb")
        with tc.tile_pool(name="psumC0", bufs=2, space="PSUM") as psumC0:
            for m in range(M):
                pc = psumC0.tile([P, 1], F32, name="pc")
                for k in range(K):
                    nc.tensor.matmul(
                        pc[:], lhsT=wT[:, k, m * P:(m + 1) * P], rhs=ms[:, k:k + 1],
                        start=(k == 0), stop=(k == K - 1),
                    )
                nc.scalar.activation(c_sb[:, m:m + 1], pc[:], AF.Copy, scale=-1.0)

        # ---- phase B: hc.T = W' @ x.T + c ; square with accumulation ----
        # (m, n-half) groups of 4 psums: stationary reused across the 4-n inner
        # loop; DVE evacuates (+c) while ACT squares -> parallel short tails.
        sqacc = small.tile([P, M, NB], F32, name="sqacc")
        with tc.tile_pool(name="psumB", bufs=2, space="PSUM") as psumB:
            for half in range(2):
                for m in range(M):
                    phs = [psumB.tile([P, 512], F32, name=f"ph{n}")
                           for n in range(4)]
                    for k in range(K):
                        for n in range(4):
                            nn = 4 * half + n
                            nc.tensor.matmul(
                                phs[n][:], lhsT=wpT[:, k, m * P:(m + 1) * P],
                                rhs=xT[:, k, 512 * nn:512 * (nn + 1)],
                                start=(k == 0), stop=(k == K - 1),
                            )
                    for n in range(4):
                        nn = 4 * half + n
                        sl = slice(512 * nn, 512 * (nn + 1))
                        nc.vector.tensor_scalar_add(hc[m][:, sl], phs[n][:],
                                                    c_sb[:, m:m + 1])
                        nc.scalar.activation(
                            hcsq[m][:, sl], hc[m][:, sl], AF.Square,
                            accum_out=sqacc[:, m, nn:nn + 1])

        sq_loc = small.tile([P, M], F32, name="sqloc")
        nc.vector.tensor_reduce(
            out=sq_loc[:], in_=sqacc[:], axis=mybir.AxisListType.X, op=ALU.add)

        if phases == "AB":
            nc.sync.dma_start(out_ap[0:P, 0:M], sq_loc[:])
            return
        # ---- AllReduce #2 ----
        ar2i = dram.tile([P, M], F32, name="ar2i")
        ar2o = dram.tile([P, M], F32, name="ar2o")
        nc.sync.dma_start(ar2i[:], sq_loc[:])
        nc.gpsimd.collective_compute(
            "AllReduce", ALU.add,
            replica_groups=[list(range(N_CORES))],
            ins=[ar2i.opt()], outs=[ar2o.opt()],
        )
        sq_g = small.tile([P, M], F32, name="sqg")
        nc.sync.dma_start(sq_g[:], ar2o[:])

        # ---- BN2 scale + template prep ----
        s2 = small.tile([P, M], F32, name="s2")
        s2sq = small.tile([P, M], BF16, name="s2sq")
        sb2 = small.tile([P, M], BF16, name="sb2")
        nc.vector.tensor_scalar_mul(s2[:], sq_g[:], 1.0 / B_GLOB)
        nc.scalar.activation(s2[:], s2[:], AF.Sqrt, bias=eps_t[:])
        nc.vector.reciprocal(s2[:], s2[:])
        nc.vector.tensor_mul(s2[:], s2[:], g2_sb[:])
        nc.vector.tensor_mul(s2sq[:], s2[:], s2[:])
        nc.vector.tensor_mul(sb2[:], s2[:], b2_sb[:])
        nc.vector.tensor_scalar_mul(sb2[:], sb2[:], 2.0)
        tp = small.tile([P, M, 33], BF16, name="tp")
        nc.vector.memset(tp.rearrange("p a b -> p (a b)"), 0.0)
        for m in range(M):
            nc.vector.tensor_scalar_mul(
                tp[:, m, 0:C], t_sb[:, m, :], s2[:, m:m + 1])
            nc.vector.tensor_copy(tp[:, m, 32:33], sb2[:, m:m + 1])


        # ---- phase C: dots + norms (dense on PE), then D: transpose + store ----
        dotc = pers.tile([C, BSH], F32, name="dotc")
        one1 = small.tile([1, 1], F32, name="one1")
        nc.vector.memset(one1[:], 1.0)
        inv_nat = small.tile([P, NT], F32, name="invnat")
        out_nat = pers.tile([P, NT, C], F32, name="outnat")
        # partition p holds batch rows 32p+i -> contiguous per-partition output runs
        out_r = out_ap.rearrange("(p g j) c -> g p j c", g=NT // 4, j=4)
        with (
            tc.tile_pool(name="psumD", bufs=2, space="PSUM") as psumD,
            tc.tile_pool(name="psumO", bufs=2, space="PSUM") as psumO,
        ):
            nxs_all = small.tile([1, BSH], F32, name="nxsall")
            for n in range(NB):
                sl = slice(512 * n, 512 * (n + 1))
                pd = psumD.tile([C, 512], F32, name="pd")
                for m in range(M):
                    nc.tensor.matmul(
                        pd[:], lhsT=tp[:, m, 0:C], rhs=hc[m][:, sl],
                        start=(m == 0), stop=(m == M - 1))
                nc.vector.tensor_scalar(
                    out=dotc[:, sl], in0=pd[:], scalar1=invnt[:],
                    scalar2=kapi[:], op0=ALU.mult, op1=ALU.add)
                pnx = psumD.tile([1, 512], F32, name="pnx")
                first = True
                for m in range(M):
                    nc.tensor.matmul(
                        pnx[:], lhsT=s2sq[:, m:m + 1], rhs=hcsq[m][:, sl],
                        start=first, stop=False)
                    first = False
                    nc.tensor.matmul(
                        pnx[:], lhsT=tp[:, m, 32:33], rhs=hc[m][:, sl],
                        start=False, stop=(m == M - 1))
                nc.scalar.activation(nxs_all[:, sl], pnx[:], AF.Sqrt, bias=bsq[:])
            # transpose ||h|| [1,4096] -> [128,32] via K=1 matmuls (after the MM
            # sweep so PE never stalls on the ACT sqrt round-trip), then recip
            for n in range(NB):
                pt = psumO.tile([P, 4], F32, name="pt")
                for j in range(4):
                    c0 = 512 * n + 128 * j
                    nc.tensor.matmul(
                        pt[:, j:j + 1], lhsT=nxs_all[:, c0:c0 + P],
                        rhs=one1[:], start=True, stop=True)
                nc.vector.reciprocal(inv_nat[:, 4 * n:4 * (n + 1)], pt[:])
            for n in range(NB):
                po = psumO.tile([P, 4, C], F32, name="po")
                for j in range(4):
                    i = 4 * n + j
                    nc.tensor.transpose(
                        po[:, j, :], dotc[:, i * P:(i + 1) * P], ident16[:])
                nc.vector.tensor_tensor(
                    out_nat[:, 4 * n:4 * (n + 1), :], po[:],
                    inv_nat[:, 4 * n:4 * (n + 1), None].to_broadcast((P, 4, C)),
                    ALU.mult)
                nc.sync.dma_start(out_r[n], out_nat[:, 4 * n:4 * (n + 1), :])


def kernel(**inputs):
    x = np.ascontiguousarray(np.asarray(inputs["x"], dtype=np.float32))
    g1 = np.ascontiguousarray(np.asarray(inputs["bn1_gamma"], dtype=np.float32))
    w = np.ascontiguousarray(np.asarray(inputs["W"], dtype=np.float32))
    g2 = np.ascontiguousarray(np.asarray(inputs["bn2_gamma"], dtype=np.float32))
    b2 = np.ascontiguousarray(np.asarray(inputs["bn2_beta"], dtype=np.float32))
    tmpl = np.ascontiguousarray(
        np.asarray(inputs["template"], dtype=np.float32).reshape(H, C))

    if "nc" not in _CACHED:
        _CACHED["nc"] = _build()
    nc = _CACHED["nc"]

    in_maps = []
    for c in range(N_CORES):
        in_maps.append({
            "x": x[c * BSH:(c + 1) * BSH],
            "bn1_gamma": g1,
            "W": w,
            "bn2_gamma": g2,
            "bn2_beta": b2,
            "template": tmpl,
        })
    res = run_bass_kernel_spmd(nc, in_maps, list(range(N_CORES)))
    out = np.concatenate([res.results[c]["out"] for c in range(N_CORES)], axis=0)
    _CACHED["exec_time_ns"] = res.exec_time_ns
    return out


if __name__ == "__main__":
    rng = np.random.default_rng(0)
    ins = {
        "x": rng.standard_normal((B_GLOB, D), dtype=np.float32),
        "bn1_gamma": np.ones(D, np.float32),
        "bn1_beta": np.zeros(D, np.float32),
        "W": rng.uniform(-1 / 32, 1 / 32, (H, D)).astype(np.float32),
        "b": rng.uniform(-1 / 32, 1 / 32, H).astype(np.float32),
        "bn2_gamma": np.ones(H, np.float32),
        "bn2_beta": np.zeros(H, np.float32),
        "template": rng.standard_normal((1, H, C), dtype=np.float32),
    }
    out = kernel(**ins)
    print("out", out.shape, out.dtype, float(np.abs(out).max()))
